# revision 39
# baseline (speedup 1.0000x reference)
"""Trainium2 Bass kernel for CharToWord bi-GRU + attention pooling,
data-parallel over 8 NeuronCores (words sharded, params replicated).

Design (vs the original grouped-GRU kernel):
- words grouped by sequence length; each core gets an equal slice of every
  group; two length groups run as interleaved chains; per-pair equalized
  exact widths NW_L (multiple of 8) instead of a global 256
- char embedding + input projection folded into per-char tables; one-hot +
  pre-gathered n-gate inputs arrive as one merged DMA per pair-step
- r/z gate sigmoid split into two Act ops so the r-dependent n-gate path
  starts earlier; n-gate recurrent bias folded into the t1 multiply via
  scalar_tensor_tensor (drops the rank-1 bias matmuls from the PE)
- per pair-step the two chains are emitted PHASE-SPLIT (A-mms+sigs,
  B-mms+sigs, A-mid, B-mid, ...) so one chain's ready work is not stuck
  behind the other's dependency-stalled ops in the strict-FIFO engines
- HAM keep-warm filler matmuls into dead PSUM tails ahead of the
  stall-prone recurrent MMs (the PE clock-gate otherwise idles at 1.2GHz
  ~70% of the time); fillers must NOT be interleaved inside an open
  PSUM accumulation group of the same tile (that corrupts results)
- attention scores computed TRANSPOSED ([words, T]) via pj-as-lhsT matmuls
  so the softmax exp-trick + denominator reduction run on the free axis;
  e transposed back with one dma_start_transpose per 128-word chunk
- attention weighted sum: one batched broadcast DMA of e per group (via a
  small DRAM scratch), DVE multiplies, PE identity-matmul accumulation in
  a PSUM bank; final divide + scatter to word order on the host
- attention emitted as a GENERATOR and drained ~3 units per GRU step of
  the NEXT pair, keeping the engine queues fed with ready work while the
  recurrent MMs wait on the previous state
- all elementwise work on DVE: gpsimd TensorTensor measured ~2.4x slower
  on HW than the simulator models, and it contends with DVE's SBUF port
"""
import sys
for p in ('/opt/trn_rl_repo', '/root/.axon_site/_ro/trn_rl_repo'):
    if p not in sys.path:
        sys.path.insert(0, p)

import numpy as np
import ml_dtypes

import concourse.bacc as bacc
import concourse.mybir as mybir
from concourse import tile

F32 = mybir.dt.float32
BF16 = mybir.dt.bfloat16
AF = mybir.ActivationFunctionType
OP = mybir.AluOpType
AX = mybir.AxisListType

VOCAB, EMB, H, C, T = 128, 64, 128, 128, 20
NCORES = 8
SCOFF = 448  # scT column offset inside the shared auxsc psum bank


def build_nc_v4(schedule, NWs, repeat=1, st_bufs=2, oh_bufs=4, work_bufs=5):
    G = len(schedule)
    W2s = [2 * nw for nw in NWs]
    # comb layout: pair-step blocks (chain A block then chain B block)
    gis_ = list(range(G))
    pairs = [(gis_[i], gis_[i + 1]) if i + 1 < G else (gis_[i], None)
             for i in range(0, G, 2)]
    comb_cols = {}
    off = 0
    for gA, gB in pairs:
        LA = schedule[gA]
        LB = schedule[gB] if gB is not None else 0
        for t in range(max(LA, LB)):
            comb_cols[('p', gA, t)] = off  # pair-step block start
            if t < LA:
                comb_cols[(gA, t)] = off
                off += 2 * W2s[gA]
            if t < LB:
                comb_cols[(gB, t)] = off
                off += 2 * W2s[gB]
    comb_total = off
    acc_offs = np.cumsum([0] + W2s).tolist()
    acc_total = int(acc_offs[-1])

    nc = bacc.Bacc(None, target_bir_lowering=False, debug=False)

    comb_d = nc.dram_tensor("comb", [VOCAB, comb_total], BF16, kind="ExternalInput")
    ef_d = nc.dram_tensor("ef", [VOCAB, 2 * H], BF16, kind="ExternalInput")
    eb_d = nc.dram_tensor("eb", [VOCAB, 2 * H], BF16, kind="ExternalInput")
    wtf_d = nc.dram_tensor("wtf", [H, 3 * H], BF16, kind="ExternalInput")
    wtb_d = nc.dram_tensor("wtb", [H, 3 * H], BF16, kind="ExternalInput")
    wpt_d = nc.dram_tensor("wpt", [H, 2 * C], BF16, kind="ExternalInput")
    ctx_d = nc.dram_tensor("ctxv", [C, 1], BF16, kind="ExternalInput")
    bp_d = nc.dram_tensor("bp", [C, 1], F32, kind="ExternalInput")
    bhnfc_d = nc.dram_tensor("bhnfc", [H, 1], F32, kind="ExternalInput")
    bhnbc_d = nc.dram_tensor("bhnbc", [H, 1], F32, kind="ExternalInput")
    iden_d = nc.dram_tensor("iden", [H, H], BF16, kind="ExternalInput")

    acc_d = nc.dram_tensor("acc", [H, acc_total], F32, kind="ExternalOutput")
    den_d = nc.dram_tensor("den", [128, 2 * G], F32, kind="ExternalOutput")
    e_scr_d = nc.dram_tensor("e_scratch", [G, T, 224], BF16)

    with tile.TileContext(nc) as tc:
        with (
            tc.tile_pool(name="const", bufs=1) as cpool,
            tc.tile_pool(name="oh", bufs=oh_bufs) as ohpool,
            tc.tile_pool(name="state", bufs=st_bufs) as stpool,
            tc.tile_pool(name="work", bufs=work_bufs) as wpool,
            tc.tile_pool(name="att", bufs=2) as apool,
            tc.tile_pool(name="ps", bufs=1, space="PSUM") as pspool,
        ):
            ef = cpool.tile([VOCAB, 2 * H], BF16, tag="ef")
            eb = cpool.tile([VOCAB, 2 * H], BF16, tag="eb")
            wtf = cpool.tile([H, 3 * H], BF16, tag="wtf")
            wtb = cpool.tile([H, 3 * H], BF16, tag="wtb")
            wpt = cpool.tile([H, 2 * C], BF16, tag="wpt")
            ctxv = cpool.tile([C, 1], BF16, tag="ctxv")
            bp = cpool.tile([C, 1], F32, tag="bp")
            bhnf_col = cpool.tile([H, 1], F32, tag="bhnf_col")
            bhnb_col = cpool.tile([H, 1], F32, tag="bhnb_col")
            ident = cpool.tile([H, H], BF16, tag="ident")
            for sb, dr in ((ef, ef_d), (eb, eb_d), (wtf, wtf_d), (wtb, wtb_d),
                           (wpt, wpt_d), (ctxv, ctx_d), (bp, bp_d),
                           (bhnf_col, bhnfc_d), (bhnb_col, bhnbc_d),
                           (ident, iden_d)):
                nc.sync.dma_start(sb[:], dr[:])

            den_all = cpool.tile([128, 2 * G], F32, tag="den_all")
            nc.gpsimd.memset(den_all[:], 0.0)

            mm = nc.tensor.matmul

            # HAM keep-warm: the dependency stalls between matmul bursts
            # leave the PE clock-gated at K=4/8 (1.2 GHz) ~70% of the time.
            # Filler matmuls into unused PSUM tail regions, queued ahead of
            # the stall-prone recurrent MMs, keep the activity monitor busy
            # so real MMs run at 2.4 GHz. PSUM is full, so the fillers write
            # into dead tails of live tiles (prz cols 928:1024, aux 488:512).
            def emit_fill(dst, n):
                for _ in range(n):
                    w = dst.shape[-1]
                    mm(dst, ef[:, 0:H], ef[:, 0:w], start=True, stop=True)

            def emit_mms(cn, gi, t, prevt, comb_t, prz):
                """gate matmuls for one chain; returns (prz, phn)"""
                NW = NWs[gi]
                W2 = 2 * NW
                oh_t = comb_t[:, 0:W2]
                b0 = 0
                b1 = 512
                if t == 0:
                    mm(prz[:, b0:b0 + NW], ef[:, 0:H], oh_t[:, 0:NW], start=True, stop=True)
                    mm(prz[:, b0 + NW:b0 + W2], eb[:, 0:H], oh_t[:, NW:W2], start=True, stop=True)
                    mm(prz[:, b1:b1 + NW], ef[:, H:2 * H], oh_t[:, 0:NW], start=True, stop=True)
                    mm(prz[:, b1 + NW:b1 + W2], eb[:, H:2 * H], oh_t[:, NW:W2], start=True, stop=True)
                    return prz, None
                prevf = prevt[:, 0:NW]
                prevb = prevt[:, NW:W2]
                mm(prz[:, b0:b0 + NW], ef[:, 0:H], oh_t[:, 0:NW], start=True, stop=False)
                mm(prz[:, b0:b0 + NW], wtf[:, 0:H], prevf, start=False, stop=True)
                mm(prz[:, b0 + NW:b0 + W2], eb[:, 0:H], oh_t[:, NW:W2], start=True, stop=False)
                mm(prz[:, b0 + NW:b0 + W2], wtb[:, 0:H], prevb, start=False, stop=True)
                mm(prz[:, b1:b1 + NW], ef[:, H:2 * H], oh_t[:, 0:NW], start=True, stop=False)
                mm(prz[:, b1:b1 + NW], wtf[:, H:2 * H], prevf, start=False, stop=True)
                mm(prz[:, b1 + NW:b1 + W2], eb[:, H:2 * H], oh_t[:, NW:W2], start=True, stop=False)
                mm(prz[:, b1 + NW:b1 + W2], wtb[:, H:2 * H], prevb, start=False, stop=True)
                phn = pspool.tile([H, 448], F32, tag=f"hn{cn}", name=f"phn{cn}_{gi}_{t}")
                mm(phn[:, 0:NW], wtf[:, 2 * H:3 * H], prevf, start=True, stop=True)
                mm(phn[:, NW:W2], wtb[:, 2 * H:3 * H], prevb, start=True, stop=True)
                return prz, phn

            def emit_mid(cn, gi, t, rz, sg, xn_t, phn):
                """t1 + sg for one chain (pre-tanh)."""
                NW = NWs[gi]
                W2 = 2 * NW
                rzr = rz[:, 0:W2]
                t1 = wpool.tile([H, 448], BF16, tag=f"t1{cn}")
                if t == 0:
                    nc.vector.tensor_scalar(t1[:, 0:NW], rzr[:, 0:NW], bhnf_col[:, 0:1],
                                            None, op0=OP.mult)
                    nc.vector.tensor_scalar(t1[:, NW:W2], rzr[:, NW:W2], bhnb_col[:, 0:1],
                                            None, op0=OP.mult)
                    nc.vector.tensor_tensor(sg[:, 0:W2], t1[:, 0:W2], xn_t, op=OP.add)
                else:
                    # t1 = (phn + bhn) * r  — bias folded via scalar_tensor_tensor
                    nc.vector.scalar_tensor_tensor(
                        t1[:, 0:NW], phn[:, 0:NW], bhnf_col[:, 0:1], rzr[:, 0:NW],
                        op0=OP.add, op1=OP.mult)
                    nc.vector.scalar_tensor_tensor(
                        t1[:, NW:W2], phn[:, NW:W2], bhnb_col[:, 0:1], rzr[:, NW:W2],
                        op0=OP.add, op1=OP.mult)
                    nc.vector.tensor_tensor(sg[:, 0:W2], t1[:, 0:W2], xn_t, op=OP.add)

            def emit_tail(cn, gi, t, prevt, sts, rz, ntr):
                """post-tanh DVE tail for one chain."""
                NW = NWs[gi]
                W2 = 2 * NW
                rzz = rz[:, W2:2 * W2]
                if t == 0:
                    et = wpool.tile([H, 448], BF16, tag=f"et{cn}")
                    nc.vector.tensor_tensor(et[:, 0:W2], rzz, ntr, op=OP.mult)
                    nc.vector.tensor_tensor(sts[t][:], ntr, et[:, 0:W2], op=OP.subtract)
                else:
                    dt_ = wpool.tile([H, 448], BF16, tag=f"dt{cn}")
                    nc.vector.tensor_tensor(dt_[:, 0:W2], prevt[:], ntr, op=OP.subtract)
                    et = wpool.tile([H, 448], BF16, tag=f"et{cn}")
                    nc.vector.tensor_tensor(et[:, 0:W2], rzz, dt_[:, 0:W2], op=OP.mult)
                    nc.vector.tensor_tensor(sts[t][:], ntr, et[:, 0:W2], op=OP.add)
                return sts[t]

            def att_gen(cn, gi, sts):
                """Attention for one finished group, as a generator yielding
                between ~1µs work units so it can be interleaved between the
                next pair's GRU steps (keeps engine FIFOs fed with ready work
                while the recurrent MMs wait on the previous state)."""
                L = schedule[gi]
                NW = NWs[gi]
                W2 = 2 * NW
                NHI = NW - 128
                # shared psum bank: cols 0:2NW proj aux (also reused as ebc
                # during the weighted sum), cols SCOFF.. transposed scores
                auxsc = pspool.tile([128, 512], F32, tag="auxsc", name=f"aux_{cn}{gi}")
                sclo = auxsc[:, SCOFF:SCOFF + 2 * T]
                pjt = wpool.tile([C, 2 * NW], BF16, tag=f"pj{cn}")
                for t0 in range(0, L, 2):
                    emit_fill(auxsc[:, 488:512], 1)
                    npos = min(2, L - t0)
                    for j in range(npos):
                        k = t0 + j
                        fsrc = sts[k][:, 0:NW]
                        bsrc = sts[L - 1 - k][:, NW:W2]
                        mm(auxsc[:, j * NW:j * NW + NW], wpt[:, 0:C], fsrc,
                           start=True, stop=False)
                        mm(auxsc[:, j * NW:j * NW + NW], wpt[:, C:2 * C], bsrc,
                           start=False, stop=True)
                    nc.scalar.activation(pjt[:, 0:npos * NW], auxsc[:, 0:npos * NW],
                                         AF.Tanh, bias=bp[:, 0:1])
                    for j in range(npos):
                        k = t0 + j
                        mm(sclo[:, 2 * k:2 * k + 1], pjt[:, j * NW:j * NW + 128],
                           ctxv[:, 0:1], start=True, stop=True)
                        mm(sclo[0:NHI, 2 * k + 1:2 * k + 2],
                           pjt[:, j * NW + 128:j * NW + NW],
                           ctxv[:, 0:1], start=True, stop=True)
                    yield
                # exp trick; e_wT [128 words, 128 tcols] per chunk (lo/hi)
                e_wT = apool.tile([128, 256], BF16, tag=f"ewt{cn}")
                nc.gpsimd.memset(e_wT[:], 0.0)
                th = wpool.tile([128, 2 * T], F32, tag=f"th{cn}")
                # th cols 0:L lo-chunk, T:T+L hi-chunk
                nc.scalar.activation(th[:, 0:L], sclo[:, 0:2 * L:2], AF.Tanh, scale=0.5)
                nc.scalar.activation(th[0:NHI, T:T + L], sclo[0:NHI, 1:2 * L:2],
                                     AF.Tanh, scale=0.5)
                enum = wpool.tile([128, 2 * T], F32, tag=f"enum{cn}")
                eden = wpool.tile([128, 2 * T], F32, tag=f"eden{cn}")
                erec = wpool.tile([128, 2 * T], F32, tag=f"erec{cn}")
                nc.vector.tensor_scalar_add(enum[:, 0:L], th[:, 0:L], 1.0)
                nc.vector.tensor_scalar_add(enum[0:NHI, T:T + L], th[0:NHI, T:T + L], 1.0)
                nc.vector.tensor_scalar(eden[:, 0:L], th[:, 0:L], -1.0, 1.0,
                                        op0=OP.mult, op1=OP.add)
                nc.vector.tensor_scalar(eden[0:NHI, T:T + L], th[0:NHI, T:T + L],
                                        -1.0, 1.0, op0=OP.mult, op1=OP.add)
                nc.vector.reciprocal(erec[:, 0:L], eden[:, 0:L])
                nc.vector.reciprocal(erec[0:NHI, T:T + L], eden[0:NHI, T:T + L])
                nc.vector.tensor_tensor(e_wT[:, 0:L], enum[:, 0:L], erec[:, 0:L],
                                        op=OP.mult)
                nc.vector.tensor_tensor(e_wT[0:NHI, 128:128 + L], enum[0:NHI, T:T + L],
                                        erec[0:NHI, T:T + L], op=OP.mult)
                nc.vector.reduce_sum(den_all[:, 2 * gi:2 * gi + 1], e_wT[:, 0:L], axis=AX.X)
                nc.vector.reduce_sum(den_all[0:NHI, 2 * gi + 1:2 * gi + 2],
                                     e_wT[0:NHI, 128:128 + L], axis=AX.X)
                # transpose each chunk back: [128 words, 128 t] -> [128 t, 128 w]
                e_sb = apool.tile([128, 256], BF16, tag=f"esb{cn}")
                nc.sync.dma_start_transpose(e_sb[:, 0:128], e_wT[:, 0:128])
                nc.sync.dma_start_transpose(e_sb[:, 128:256], e_wT[:, 128:256])
                # weighted sum; one batched broadcast of all e rows to all
                # 128 partitions: ebc_all[h, k, w] = e[k, w]
                nc.sync.dma_start(e_scr_d[gi, 0:L, 0:NW], e_sb[0:L, 0:NW])
                ebc_all = apool.tile([H, T * 224], BF16, tag=f"ebc{cn}", bufs=1)
                src = e_scr_d[gi, 0:L, 0:NW].partition_broadcast(H)
                nc.sync.dma_start(
                    ebc_all[:].rearrange("p (l w) -> p l w", l=T)[:, 0:L, 0:NW], src)
                yield
                acc = pspool.tile([H, W2], F32, tag="acc", name=f"acc_{cn}{gi}")
                m_ = wpool.tile([H, W2], BF16, tag=f"m{cn}")
                for k in range(L):
                    ek = ebc_all[:, k * 224:k * 224 + NW]
                    nc.vector.tensor_tensor(m_[:, 0:NW], sts[k][:, 0:NW],
                                            ek, op=OP.mult)
                    nc.vector.tensor_tensor(m_[:, NW:W2], sts[L - 1 - k][:, NW:W2],
                                            ek, op=OP.mult)
                    mm(acc[:], ident[:], m_[:], start=(k == 0), stop=(k == L - 1))
                    if k % 2 == 1:
                        yield
                acc_sb = apool.tile([H, W2], F32, tag=f"accsb{cn}")
                nc.scalar.copy(acc_sb[:], acc[:])
                nc.sync.dma_start(acc_d[:, acc_offs[gi]:acc_offs[gi] + W2], acc_sb[:])

            gis = list(range(G))
            pairs = [(gis[i], gis[i + 1]) if i + 1 < G else (gis[i], None)
                     for i in range(0, G, 2)]

            def emit_pairs():
                pending = []  # att generators from finished pairs

                def drain(n):
                    done = 0
                    while pending and done < n:
                        try:
                            next(pending[0])
                            done += 1
                        except StopIteration:
                            pending.pop(0)

                for gA, gB in pairs:
                    LA = schedule[gA]
                    LB = schedule[gB] if gB is not None else 0
                    NWp = NWs[gA]
                    W2p = 2 * NWp
                    stsA = [stpool.tile([H, 2 * NWs[gA]], BF16, tag=f"sta{t}",
                                        name=f"sta{gA}_{t}") for t in range(LA)]
                    stsB = [stpool.tile([H, 2 * NWs[gB]], BF16, tag=f"stb{t}",
                                        name=f"stb{gA}_{t}") for t in range(LB)]
                    pA = pB = None
                    for t in range(max(LA, LB)):
                        a_on = t < LA
                        b_on = t < LB
                        wA = 2 * W2p if a_on else 0
                        wB = 2 * W2p if b_on else 0
                        comb_t = ohpool.tile([VOCAB, 4 * 448], BF16, tag="comb",
                                             name=f"comb_{gA}_{t}")
                        base = comb_cols[('p', gA, t)]
                        nc.sync.dma_start(comb_t[:, 0:wA + wB],
                                          comb_d[:, base:base + wA + wB])
                        combA = comb_t[:, 0:wA]
                        combB = comb_t[:, wA:wA + wB]
                        # phase-split emission: both chains' same-phase ops
                        # are adjacent in each engine queue, so chain B's
                        # ready work is never stuck behind chain A's
                        # dependency-stalled ops (strict-FIFO engines).
                        if a_on:
                            przA_t = pspool.tile([H, 1024], F32, tag="prza",
                                                 name=f"prza_{gA}_{t}")
                            emit_fill(przA_t[:, 928:1024], 3)
                            przA, phA = emit_mms('a', gA, t, pA, combA, przA_t)
                            rzA = wpool.tile([H, 2 * 448], BF16, tag="rza")
                            nc.scalar.activation(rzA[:, 0:W2p], przA[:, 0:W2p], AF.Sigmoid)
                            nc.scalar.activation(rzA[:, W2p:2 * W2p], przA[:, 512:512 + W2p],
                                                 AF.Sigmoid)
                        if b_on:
                            przB_t = pspool.tile([H, 1024], F32, tag="przb",
                                                 name=f"przb_{gB}_{t}")
                            emit_fill(przB_t[:, 928:1024], 3)
                            przB, phB = emit_mms('b', gB, t, pB, combB, przB_t)
                            rzB = wpool.tile([H, 2 * 448], BF16, tag="rzb")
                            nc.scalar.activation(rzB[:, 0:W2p], przB[:, 0:W2p], AF.Sigmoid)
                            nc.scalar.activation(rzB[:, W2p:2 * W2p], przB[:, 512:512 + W2p],
                                                 AF.Sigmoid)
                        if a_on:
                            sgA = wpool.tile([H, 448], BF16, tag="sga")
                            emit_mid('a', gA, t, rzA, sgA, combA[:, W2p:2 * W2p], phA)
                        if b_on:
                            sgB = wpool.tile([H, 448], BF16, tag="sgb")
                            emit_mid('b', gB, t, rzB, sgB, combB[:, W2p:2 * W2p], phB)
                        if a_on:
                            ntA = wpool.tile([H, 448], BF16, tag="nta")
                            nc.scalar.activation(ntA[:, 0:W2p], sgA[:, 0:W2p], AF.Tanh)
                        if b_on:
                            ntB = wpool.tile([H, 448], BF16, tag="ntb")
                            nc.scalar.activation(ntB[:, 0:W2p], sgB[:, 0:W2p], AF.Tanh)
                        if a_on:
                            pA = emit_tail('a', gA, t, pA, stsA, rzA, ntA[:, 0:W2p])
                        if b_on:
                            pB = emit_tail('b', gB, t, pB, stsB, rzB, ntB[:, 0:W2p])
                        drain(3)
                    pending.append(att_gen('a', gA, stsA))
                    if gB is not None:
                        pending.append(att_gen('b', gB, stsB))
                while pending:
                    drain(100)
            if repeat > 1:
                with tc.For_i(0, repeat, 1):
                    emit_pairs()
            else:
                emit_pairs()
            nc.sync.dma_start(den_d[:], den_all[:])
    nc.finalize()
    return nc


def prep_host_v4(chars, lens, emb, Wih_f, Whh_f, bih_f, bhh_f,
                 Wih_b, Whh_b, bih_b, bhh_b, Wp, bp, ctx):
    bf = ml_dtypes.bfloat16
    schedule = list(range(1, T + 1))
    counts = np.bincount(lens, minlength=T + 1)[1:]
    NWs = []
    for L in schedule:
        maxc = int(np.ceil(counts[L - 1] / NCORES))
        nw = max(136, ((maxc + 7) // 8) * 8)
        assert nw <= 224
        NWs.append(nw)
    for i in range(0, len(NWs) - 1, 2):  # equal widths within each pair
        m = max(NWs[i], NWs[i + 1])
        NWs[i] = NWs[i + 1] = m
    ids = [np.full((NCORES, NWs[L - 1]), -1, dtype=np.int64) for L in schedule]
    for L in schedule:
        arr = np.nonzero(lens == L)[0]
        for c in range(NCORES):
            sub = arr[c::NCORES]
            ids[L - 1][c, :len(sub)] = sub

    def make_E(Wih, bih, bhh):
        E = emb.astype(np.float64) @ Wih.T.astype(np.float64) + bih.astype(np.float64)
        E[:, 0:H] += bhh[0:H]
        E[:, H:2 * H] += bhh[H:2 * H]
        return E
    Ef3 = make_E(Wih_f, bih_f, bhh_f)
    Eb3 = make_E(Wih_b, bih_b, bhh_b)
    Ef = Ef3[:, 0:2 * H].astype(bf)
    Eb = Eb3[:, 0:2 * H].astype(bf)
    EfN = Ef3[:, 2 * H:3 * H].astype(bf)
    EbN = Eb3[:, 2 * H:3 * H].astype(bf)

    def step_block(c, gi, t):
        L = schedule[gi]
        NW = NWs[gi]
        W2 = 2 * NW
        idx = ids[gi][c].clip(0)
        ch = chars[idx]
        code_f = ch[:, t]
        code_b = ch[:, L - 1 - t]
        blk = np.zeros((VOCAB, 2 * W2), dtype=bf)
        ar = np.arange(NW)
        blk[code_f, ar] = 1
        blk[code_b, NW + ar] = 1
        blk[:, W2:W2 + NW] = EfN[code_f].T
        blk[:, W2 + NW:2 * W2] = EbN[code_b].T
        return blk

    gis_ = list(range(len(schedule)))
    pairs = [(gis_[i], gis_[i + 1]) if i + 1 < len(schedule) else (gis_[i], None)
             for i in range(0, len(schedule), 2)]
    combs = []
    for c in range(NCORES):
        blocks = []
        for gA, gB in pairs:
            LA = schedule[gA]
            LB = schedule[gB] if gB is not None else 0
            for t in range(max(LA, LB)):
                if t < LA:
                    blocks.append(step_block(c, gA, t))
                if t < LB:
                    blocks.append(step_block(c, gB, t))
        combs.append(np.concatenate(blocks, axis=1))

    wpt = Wp.T
    wpt2 = np.concatenate([wpt[0:H, :], wpt[H:2 * H, :]], axis=1)
    s0 = float(ctx[:, 0] @ np.tanh(bp))
    es0 = float(np.exp(s0))

    params = dict(
        ef=Ef, eb=Eb,
        wtf=Whh_f.T.astype(bf), wtb=Whh_b.T.astype(bf),
        wpt=wpt2.astype(bf),
        ctxv=ctx.astype(bf).reshape(C, 1), bp=bp.astype(np.float32).reshape(C, 1),
        bhnfc=bhh_f[2 * H:].astype(np.float32).reshape(H, 1),
        bhnbc=bhh_b[2 * H:].astype(np.float32).reshape(H, 1),
        iden=np.eye(H, dtype=bf),
    )
    in_maps = [dict(comb=combs[c], **params) for c in range(NCORES)]
    return schedule, NWs, in_maps, ids, es0


def post_host_v4(results, schedule, NWs, ids, es0, B):
    acc_offs = np.cumsum([0] + [2 * nw for nw in NWs]).tolist()
    out = np.zeros((B, 2 * H), dtype=np.float32)
    for c in range(NCORES):
        acc = np.asarray(results[c]["acc"], dtype=np.float32)
        den = np.asarray(results[c]["den"], dtype=np.float32)
        for gi, L in enumerate(schedule):
            NW = NWs[gi]
            idlist = ids[gi][c]
            w = np.nonzero(idlist >= 0)[0]
            if len(w) == 0:
                continue
            denw = np.empty(NW, dtype=np.float32)
            denw[0:128] = den[:, 2 * gi]
            denw[128:NW] = den[0:NW - 128, 2 * gi + 1]
            denom = denw[w] + (T - L) * es0
            a = acc[:, acc_offs[gi]:acc_offs[gi] + 2 * NW]
            out[idlist[w], 0:H] = (a[:, w] / denom).T
            out[idlist[w], H:2 * H] = (a[:, NW + w] / denom).T
    return out


_NC_CACHE = {}


def get_nc(schedule, NWs, repeat=1):
    key = (tuple(schedule), tuple(NWs), repeat)
    if key not in _NC_CACHE:
        _NC_CACHE[key] = build_nc_v4(schedule, NWs, repeat=repeat)
    return _NC_CACHE[key]


def kernel(**inputs):
    chars = np.asarray(inputs['padded_char_tensor'])
    lens = np.asarray(inputs['sequence_lens'])
    B = chars.shape[0]
    args = [np.asarray(inputs[k], dtype=np.float32) for k in (
        'emb', 'Wih_f', 'Whh_f', 'bih_f', 'bhh_f',
        'Wih_b', 'Whh_b', 'bih_b', 'bhh_b', 'Wp', 'bp', 'ctx')]
    schedule, NWs, in_maps, ids, es0 = prep_host_v4(
        chars.astype(np.int64), lens.astype(np.int64), *args)
    nc = get_nc(schedule, NWs)
    from concourse.bass_utils import run_bass_kernel_spmd
    res = run_bass_kernel_spmd(nc, in_maps, list(range(NCORES)))
    out = post_host_v4(res.results, schedule, NWs, ids, es0, B)
    return out.astype(np.float32)



# revision 40
# speedup vs baseline: 1.0012x; 1.0012x over previous
"""Trainium2 Bass kernel for CharToWord bi-GRU + attention pooling,
data-parallel over 8 NeuronCores (words sharded, params replicated).

Design (vs the original grouped-GRU kernel):
- words grouped by sequence length; each core gets an equal slice of every
  group; two length groups run as interleaved chains; per-pair equalized
  exact widths NW_L (multiple of 8) instead of a global 256
- char embedding + input projection folded into per-char tables; one-hot +
  pre-gathered n-gate inputs arrive as one merged DMA per pair-step
- r/z gate sigmoid split into two Act ops so the r-dependent n-gate path
  starts earlier; n-gate recurrent bias folded into the t1 multiply via
  scalar_tensor_tensor (drops the rank-1 bias matmuls from the PE)
- per pair-step the two chains are emitted PHASE-SPLIT (A-mms+sigs,
  B-mms+sigs, A-mid, B-mid, ...) so one chain's ready work is not stuck
  behind the other's dependency-stalled ops in the strict-FIFO engines
- HAM keep-warm filler matmuls into dead PSUM tails ahead of the
  stall-prone recurrent MMs (the PE clock-gate otherwise idles at 1.2GHz
  ~70% of the time); fillers must NOT be interleaved inside an open
  PSUM accumulation group of the same tile (that corrupts results)
- attention scores computed TRANSPOSED ([words, T]) via pj-as-lhsT matmuls
  so the softmax exp-trick + denominator reduction run on the free axis;
  e transposed back with one dma_start_transpose per 128-word chunk
- attention weighted sum: one batched broadcast DMA of e per group (via a
  small DRAM scratch), DVE multiplies, PE identity-matmul accumulation in
  a PSUM bank; final divide + scatter to word order on the host
- attention emitted as a GENERATOR and drained ~3 units per GRU step of
  the NEXT pair, keeping the engine queues fed with ready work while the
  recurrent MMs wait on the previous state
- all elementwise work on DVE: gpsimd TensorTensor measured ~2.4x slower
  on HW than the simulator models, and it contends with DVE's SBUF port
"""
import sys
for p in ('/opt/trn_rl_repo', '/root/.axon_site/_ro/trn_rl_repo'):
    if p not in sys.path:
        sys.path.insert(0, p)

import numpy as np
import ml_dtypes

import concourse.bacc as bacc
import concourse.mybir as mybir
from concourse import tile

F32 = mybir.dt.float32
BF16 = mybir.dt.bfloat16
AF = mybir.ActivationFunctionType
OP = mybir.AluOpType
AX = mybir.AxisListType

VOCAB, EMB, H, C, T = 128, 64, 128, 128, 20
NCORES = 8
SCOFF = 448  # scT column offset inside the shared auxsc psum bank


def build_nc_v4(schedule, NWs, repeat=1, st_bufs=2, oh_bufs=4, work_bufs=5):
    G = len(schedule)
    W2s = [2 * nw for nw in NWs]
    # comb layout: pair-step blocks (chain A block then chain B block)
    gis_ = list(range(G))
    pairs = [(gis_[i], gis_[i + 1]) if i + 1 < G else (gis_[i], None)
             for i in range(0, G, 2)]
    comb_cols = {}
    off = 0
    for gA, gB in pairs:
        LA = schedule[gA]
        LB = schedule[gB] if gB is not None else 0
        for t in range(max(LA, LB)):
            comb_cols[('p', gA, t)] = off  # pair-step block start
            if t < LA:
                comb_cols[(gA, t)] = off
                off += 2 * W2s[gA]
            if t < LB:
                comb_cols[(gB, t)] = off
                off += 2 * W2s[gB]
    comb_total = off
    acc_offs = np.cumsum([0] + W2s).tolist()
    acc_total = int(acc_offs[-1])

    nc = bacc.Bacc(None, target_bir_lowering=False, debug=False)

    comb_d = nc.dram_tensor("comb", [VOCAB, comb_total], BF16, kind="ExternalInput")
    ef_d = nc.dram_tensor("ef", [VOCAB, 2 * H], BF16, kind="ExternalInput")
    eb_d = nc.dram_tensor("eb", [VOCAB, 2 * H], BF16, kind="ExternalInput")
    wtf_d = nc.dram_tensor("wtf", [H, 3 * H], BF16, kind="ExternalInput")
    wtb_d = nc.dram_tensor("wtb", [H, 3 * H], BF16, kind="ExternalInput")
    wpt_d = nc.dram_tensor("wpt", [H, 2 * C], BF16, kind="ExternalInput")
    ctx_d = nc.dram_tensor("ctxv", [C, 1], BF16, kind="ExternalInput")
    bp_d = nc.dram_tensor("bp", [C, 1], F32, kind="ExternalInput")
    bhnfc_d = nc.dram_tensor("bhnfc", [H, 1], F32, kind="ExternalInput")
    bhnbc_d = nc.dram_tensor("bhnbc", [H, 1], F32, kind="ExternalInput")
    iden_d = nc.dram_tensor("iden", [H, H], BF16, kind="ExternalInput")

    acc_d = nc.dram_tensor("acc", [H, acc_total], F32, kind="ExternalOutput")
    den_d = nc.dram_tensor("den", [128, 2 * G], F32, kind="ExternalOutput")
    e_scr_d = nc.dram_tensor("e_scratch", [G, T, 224], BF16)

    with tile.TileContext(nc) as tc:
        with (
            tc.tile_pool(name="const", bufs=1) as cpool,
            tc.tile_pool(name="oh", bufs=oh_bufs) as ohpool,
            tc.tile_pool(name="state", bufs=st_bufs) as stpool,
            tc.tile_pool(name="work", bufs=work_bufs) as wpool,
            tc.tile_pool(name="att", bufs=2) as apool,
            tc.tile_pool(name="ps", bufs=1, space="PSUM") as pspool,
        ):
            ef = cpool.tile([VOCAB, 2 * H], BF16, tag="ef")
            eb = cpool.tile([VOCAB, 2 * H], BF16, tag="eb")
            wtf = cpool.tile([H, 3 * H], BF16, tag="wtf")
            wtb = cpool.tile([H, 3 * H], BF16, tag="wtb")
            wpt = cpool.tile([H, 2 * C], BF16, tag="wpt")
            ctxv = cpool.tile([C, 1], BF16, tag="ctxv")
            bp = cpool.tile([C, 1], F32, tag="bp")
            bhnf_col = cpool.tile([H, 1], F32, tag="bhnf_col")
            bhnb_col = cpool.tile([H, 1], F32, tag="bhnb_col")
            ident = cpool.tile([H, H], BF16, tag="ident")
            for sb, dr in ((ef, ef_d), (eb, eb_d), (wtf, wtf_d), (wtb, wtb_d),
                           (wpt, wpt_d), (ctxv, ctx_d), (bp, bp_d),
                           (bhnf_col, bhnfc_d), (bhnb_col, bhnbc_d),
                           (ident, iden_d)):
                nc.sync.dma_start(sb[:], dr[:])

            den_all = cpool.tile([128, 2 * G], F32, tag="den_all")
            nc.gpsimd.memset(den_all[:], 0.0)

            mm = nc.tensor.matmul

            # HAM keep-warm: the dependency stalls between matmul bursts
            # leave the PE clock-gated at K=4/8 (1.2 GHz) ~70% of the time.
            # Filler matmuls into unused PSUM tail regions, queued ahead of
            # the stall-prone recurrent MMs, keep the activity monitor busy
            # so real MMs run at 2.4 GHz. PSUM is full, so the fillers write
            # into dead tails of live tiles (prz cols 928:1024, aux 488:512).
            def emit_fill(dst, n):
                for _ in range(n):
                    w = dst.shape[-1]
                    mm(dst, ef[:, 0:H], ef[:, 0:w], start=True, stop=True)

            def emit_mms(cn, gi, t, prevt, comb_t, prz):
                """gate matmuls for one chain; returns (prz, phn)"""
                NW = NWs[gi]
                W2 = 2 * NW
                oh_t = comb_t[:, 0:W2]
                b0 = 0
                b1 = 512
                if t == 0:
                    mm(prz[:, b0:b0 + NW], ef[:, 0:H], oh_t[:, 0:NW], start=True, stop=True)
                    mm(prz[:, b0 + NW:b0 + W2], eb[:, 0:H], oh_t[:, NW:W2], start=True, stop=True)
                    mm(prz[:, b1:b1 + NW], ef[:, H:2 * H], oh_t[:, 0:NW], start=True, stop=True)
                    mm(prz[:, b1 + NW:b1 + W2], eb[:, H:2 * H], oh_t[:, NW:W2], start=True, stop=True)
                    return prz, None
                prevf = prevt[:, 0:NW]
                prevb = prevt[:, NW:W2]
                mm(prz[:, b0:b0 + NW], ef[:, 0:H], oh_t[:, 0:NW], start=True, stop=False)
                mm(prz[:, b0:b0 + NW], wtf[:, 0:H], prevf, start=False, stop=True)
                mm(prz[:, b0 + NW:b0 + W2], eb[:, 0:H], oh_t[:, NW:W2], start=True, stop=False)
                mm(prz[:, b0 + NW:b0 + W2], wtb[:, 0:H], prevb, start=False, stop=True)
                mm(prz[:, b1:b1 + NW], ef[:, H:2 * H], oh_t[:, 0:NW], start=True, stop=False)
                mm(prz[:, b1:b1 + NW], wtf[:, H:2 * H], prevf, start=False, stop=True)
                mm(prz[:, b1 + NW:b1 + W2], eb[:, H:2 * H], oh_t[:, NW:W2], start=True, stop=False)
                mm(prz[:, b1 + NW:b1 + W2], wtb[:, H:2 * H], prevb, start=False, stop=True)
                phn = pspool.tile([H, 448], F32, tag=f"hn{cn}", name=f"phn{cn}_{gi}_{t}")
                mm(phn[:, 0:NW], wtf[:, 2 * H:3 * H], prevf, start=True, stop=True)
                mm(phn[:, NW:W2], wtb[:, 2 * H:3 * H], prevb, start=True, stop=True)
                return prz, phn

            def emit_mid(cn, gi, t, rz, sg, xn_t, phn):
                """t1 + sg for one chain (pre-tanh)."""
                NW = NWs[gi]
                W2 = 2 * NW
                rzr = rz[:, 0:W2]
                t1 = wpool.tile([H, 448], BF16, tag=f"t1{cn}")
                if t == 0:
                    nc.vector.tensor_scalar(t1[:, 0:NW], rzr[:, 0:NW], bhnf_col[:, 0:1],
                                            None, op0=OP.mult)
                    nc.vector.tensor_scalar(t1[:, NW:W2], rzr[:, NW:W2], bhnb_col[:, 0:1],
                                            None, op0=OP.mult)
                    nc.vector.tensor_tensor(sg[:, 0:W2], t1[:, 0:W2], xn_t, op=OP.add)
                else:
                    # t1 = (phn + bhn) * r  — bias folded via scalar_tensor_tensor
                    nc.vector.scalar_tensor_tensor(
                        t1[:, 0:NW], phn[:, 0:NW], bhnf_col[:, 0:1], rzr[:, 0:NW],
                        op0=OP.add, op1=OP.mult)
                    nc.vector.scalar_tensor_tensor(
                        t1[:, NW:W2], phn[:, NW:W2], bhnb_col[:, 0:1], rzr[:, NW:W2],
                        op0=OP.add, op1=OP.mult)
                    nc.vector.tensor_tensor(sg[:, 0:W2], t1[:, 0:W2], xn_t, op=OP.add)

            def emit_tail(cn, gi, t, prevt, sts, rz, ntr):
                """post-tanh DVE tail for one chain."""
                NW = NWs[gi]
                W2 = 2 * NW
                rzz = rz[:, W2:2 * W2]
                if t == 0:
                    et = wpool.tile([H, 448], BF16, tag=f"et{cn}")
                    nc.vector.tensor_tensor(et[:, 0:W2], rzz, ntr, op=OP.mult)
                    nc.vector.tensor_tensor(sts[t][:], ntr, et[:, 0:W2], op=OP.subtract)
                else:
                    dt_ = wpool.tile([H, 448], BF16, tag=f"dt{cn}")
                    nc.vector.tensor_tensor(dt_[:, 0:W2], prevt[:], ntr, op=OP.subtract)
                    et = wpool.tile([H, 448], BF16, tag=f"et{cn}")
                    nc.vector.tensor_tensor(et[:, 0:W2], rzz, dt_[:, 0:W2], op=OP.mult)
                    nc.vector.tensor_tensor(sts[t][:], ntr, et[:, 0:W2], op=OP.add)
                return sts[t]

            def att_gen(cn, gi, sts):
                """Attention for one finished group, as a generator yielding
                between ~1µs work units so it can be interleaved between the
                next pair's GRU steps (keeps engine FIFOs fed with ready work
                while the recurrent MMs wait on the previous state)."""
                L = schedule[gi]
                NW = NWs[gi]
                W2 = 2 * NW
                NHI = NW - 128
                # shared psum bank: cols 0:2NW proj aux (also reused as ebc
                # during the weighted sum), cols SCOFF.. transposed scores
                auxsc = pspool.tile([128, 512], F32, tag="auxsc", name=f"aux_{cn}{gi}")
                sclo = auxsc[:, SCOFF:SCOFF + 2 * T]
                pjt = wpool.tile([C, 2 * NW], BF16, tag=f"pj{cn}")
                for t0 in range(0, L, 2):
                    emit_fill(auxsc[:, 488:512], 1)
                    npos = min(2, L - t0)
                    for j in range(npos):
                        k = t0 + j
                        fsrc = sts[k][:, 0:NW]
                        bsrc = sts[L - 1 - k][:, NW:W2]
                        mm(auxsc[:, j * NW:j * NW + NW], wpt[:, 0:C], fsrc,
                           start=True, stop=False)
                        mm(auxsc[:, j * NW:j * NW + NW], wpt[:, C:2 * C], bsrc,
                           start=False, stop=True)
                    nc.scalar.activation(pjt[:, 0:npos * NW], auxsc[:, 0:npos * NW],
                                         AF.Tanh, bias=bp[:, 0:1])
                    for j in range(npos):
                        k = t0 + j
                        mm(sclo[:, 2 * k:2 * k + 1], pjt[:, j * NW:j * NW + 128],
                           ctxv[:, 0:1], start=True, stop=True)
                        mm(sclo[0:NHI, 2 * k + 1:2 * k + 2],
                           pjt[:, j * NW + 128:j * NW + NW],
                           ctxv[:, 0:1], start=True, stop=True)
                    yield
                # exp trick; e_wT [128 words, 128 tcols] per chunk (lo/hi)
                e_wT = apool.tile([128, 256], BF16, tag=f"ewt{cn}")
                nc.gpsimd.memset(e_wT[:], 0.0)
                th = wpool.tile([128, 2 * T], F32, tag=f"th{cn}")
                # th cols 0:L lo-chunk, T:T+L hi-chunk
                nc.scalar.activation(th[:, 0:L], sclo[:, 0:2 * L:2], AF.Tanh, scale=0.5)
                nc.scalar.activation(th[0:NHI, T:T + L], sclo[0:NHI, 1:2 * L:2],
                                     AF.Tanh, scale=0.5)
                eden = wpool.tile([128, 2 * T], F32, tag=f"eden{cn}")
                erec = wpool.tile([128, 2 * T], F32, tag=f"erec{cn}")
                nc.gpsimd.tensor_scalar(eden[:, 0:L], th[:, 0:L], -1.0, 1.0,
                                        op0=OP.mult, op1=OP.add)
                nc.gpsimd.tensor_scalar(eden[0:NHI, T:T + L], th[0:NHI, T:T + L],
                                        -1.0, 1.0, op0=OP.mult, op1=OP.add)
                nc.vector.reciprocal(erec[:, 0:L], eden[:, 0:L])
                nc.vector.reciprocal(erec[0:NHI, T:T + L], eden[0:NHI, T:T + L])
                # e = (1 + th) * erec fused via scalar_tensor_tensor
                nc.vector.scalar_tensor_tensor(e_wT[:, 0:L], th[:, 0:L], 1.0,
                                               erec[:, 0:L], op0=OP.add, op1=OP.mult)
                nc.vector.scalar_tensor_tensor(e_wT[0:NHI, 128:128 + L],
                                               th[0:NHI, T:T + L], 1.0,
                                               erec[0:NHI, T:T + L],
                                               op0=OP.add, op1=OP.mult)
                nc.vector.reduce_sum(den_all[:, 2 * gi:2 * gi + 1], e_wT[:, 0:L], axis=AX.X)
                nc.vector.reduce_sum(den_all[0:NHI, 2 * gi + 1:2 * gi + 2],
                                     e_wT[0:NHI, 128:128 + L], axis=AX.X)
                # transpose each chunk back: [128 words, 128 t] -> [128 t, 128 w]
                e_sb = apool.tile([128, 256], BF16, tag=f"esb{cn}")
                nc.sync.dma_start_transpose(e_sb[:, 0:128], e_wT[:, 0:128])
                nc.sync.dma_start_transpose(e_sb[:, 128:256], e_wT[:, 128:256])
                # weighted sum; one batched broadcast of all e rows to all
                # 128 partitions: ebc_all[h, k, w] = e[k, w]
                nc.sync.dma_start(e_scr_d[gi, 0:L, 0:NW], e_sb[0:L, 0:NW])
                ebc_all = apool.tile([H, T * 224], BF16, tag=f"ebc{cn}", bufs=1)
                src = e_scr_d[gi, 0:L, 0:NW].partition_broadcast(H)
                nc.sync.dma_start(
                    ebc_all[:].rearrange("p (l w) -> p l w", l=T)[:, 0:L, 0:NW], src)
                yield
                acc = pspool.tile([H, W2], F32, tag="acc", name=f"acc_{cn}{gi}")
                m_ = wpool.tile([H, W2], BF16, tag=f"m{cn}")
                for k in range(L):
                    ek = ebc_all[:, k * 224:k * 224 + NW]
                    nc.vector.tensor_tensor(m_[:, 0:NW], sts[k][:, 0:NW],
                                            ek, op=OP.mult)
                    nc.vector.tensor_tensor(m_[:, NW:W2], sts[L - 1 - k][:, NW:W2],
                                            ek, op=OP.mult)
                    mm(acc[:], ident[:], m_[:], start=(k == 0), stop=(k == L - 1))
                    if k % 2 == 1:
                        yield
                acc_sb = apool.tile([H, W2], F32, tag=f"accsb{cn}")
                nc.scalar.copy(acc_sb[:], acc[:])
                nc.sync.dma_start(acc_d[:, acc_offs[gi]:acc_offs[gi] + W2], acc_sb[:])

            gis = list(range(G))
            pairs = [(gis[i], gis[i + 1]) if i + 1 < G else (gis[i], None)
                     for i in range(0, G, 2)]

            def emit_pairs():
                pending = []  # att generators from finished pairs

                def drain(n):
                    done = 0
                    while pending and done < n:
                        try:
                            next(pending[0])
                            done += 1
                        except StopIteration:
                            pending.pop(0)

                for gA, gB in pairs:
                    LA = schedule[gA]
                    LB = schedule[gB] if gB is not None else 0
                    NWp = NWs[gA]
                    W2p = 2 * NWp
                    stsA = [stpool.tile([H, 2 * NWs[gA]], BF16, tag=f"sta{t}",
                                        name=f"sta{gA}_{t}") for t in range(LA)]
                    stsB = [stpool.tile([H, 2 * NWs[gB]], BF16, tag=f"stb{t}",
                                        name=f"stb{gA}_{t}") for t in range(LB)]
                    pA = pB = None
                    for t in range(max(LA, LB)):
                        a_on = t < LA
                        b_on = t < LB
                        wA = 2 * W2p if a_on else 0
                        wB = 2 * W2p if b_on else 0
                        comb_t = ohpool.tile([VOCAB, 4 * 448], BF16, tag="comb",
                                             name=f"comb_{gA}_{t}")
                        base = comb_cols[('p', gA, t)]
                        nc.sync.dma_start(comb_t[:, 0:wA + wB],
                                          comb_d[:, base:base + wA + wB])
                        combA = comb_t[:, 0:wA]
                        combB = comb_t[:, wA:wA + wB]
                        # phase-split emission: both chains' same-phase ops
                        # are adjacent in each engine queue, so chain B's
                        # ready work is never stuck behind chain A's
                        # dependency-stalled ops (strict-FIFO engines).
                        if a_on:
                            przA_t = pspool.tile([H, 1024], F32, tag="prza",
                                                 name=f"prza_{gA}_{t}")
                            emit_fill(przA_t[:, 928:1024], 3)
                            przA, phA = emit_mms('a', gA, t, pA, combA, przA_t)
                            rzA = wpool.tile([H, 2 * 448], BF16, tag="rza")
                            nc.scalar.activation(rzA[:, 0:W2p], przA[:, 0:W2p], AF.Sigmoid)
                            nc.scalar.activation(rzA[:, W2p:2 * W2p], przA[:, 512:512 + W2p],
                                                 AF.Sigmoid)
                        if b_on:
                            przB_t = pspool.tile([H, 1024], F32, tag="przb",
                                                 name=f"przb_{gB}_{t}")
                            emit_fill(przB_t[:, 928:1024], 3)
                            przB, phB = emit_mms('b', gB, t, pB, combB, przB_t)
                            rzB = wpool.tile([H, 2 * 448], BF16, tag="rzb")
                            nc.scalar.activation(rzB[:, 0:W2p], przB[:, 0:W2p], AF.Sigmoid)
                            nc.scalar.activation(rzB[:, W2p:2 * W2p], przB[:, 512:512 + W2p],
                                                 AF.Sigmoid)
                        if a_on:
                            sgA = wpool.tile([H, 448], BF16, tag="sga")
                            emit_mid('a', gA, t, rzA, sgA, combA[:, W2p:2 * W2p], phA)
                        if b_on:
                            sgB = wpool.tile([H, 448], BF16, tag="sgb")
                            emit_mid('b', gB, t, rzB, sgB, combB[:, W2p:2 * W2p], phB)
                        if a_on:
                            ntA = wpool.tile([H, 448], BF16, tag="nta")
                            nc.scalar.activation(ntA[:, 0:W2p], sgA[:, 0:W2p], AF.Tanh)
                        if b_on:
                            ntB = wpool.tile([H, 448], BF16, tag="ntb")
                            nc.scalar.activation(ntB[:, 0:W2p], sgB[:, 0:W2p], AF.Tanh)
                        if a_on:
                            pA = emit_tail('a', gA, t, pA, stsA, rzA, ntA[:, 0:W2p])
                        if b_on:
                            pB = emit_tail('b', gB, t, pB, stsB, rzB, ntB[:, 0:W2p])
                        drain(3)
                    pending.append(att_gen('a', gA, stsA))
                    if gB is not None:
                        pending.append(att_gen('b', gB, stsB))
                while pending:
                    drain(100)
            if repeat > 1:
                with tc.For_i(0, repeat, 1):
                    emit_pairs()
            else:
                emit_pairs()
            nc.sync.dma_start(den_d[:], den_all[:])
    nc.finalize()
    return nc


def prep_host_v4(chars, lens, emb, Wih_f, Whh_f, bih_f, bhh_f,
                 Wih_b, Whh_b, bih_b, bhh_b, Wp, bp, ctx):
    bf = ml_dtypes.bfloat16
    schedule = list(range(1, T + 1))
    counts = np.bincount(lens, minlength=T + 1)[1:]
    NWs = []
    for L in schedule:
        maxc = int(np.ceil(counts[L - 1] / NCORES))
        nw = max(136, ((maxc + 7) // 8) * 8)
        assert nw <= 224
        NWs.append(nw)
    for i in range(0, len(NWs) - 1, 2):  # equal widths within each pair
        m = max(NWs[i], NWs[i + 1])
        NWs[i] = NWs[i + 1] = m
    ids = [np.full((NCORES, NWs[L - 1]), -1, dtype=np.int64) for L in schedule]
    for L in schedule:
        arr = np.nonzero(lens == L)[0]
        for c in range(NCORES):
            sub = arr[c::NCORES]
            ids[L - 1][c, :len(sub)] = sub

    def make_E(Wih, bih, bhh):
        E = emb.astype(np.float64) @ Wih.T.astype(np.float64) + bih.astype(np.float64)
        E[:, 0:H] += bhh[0:H]
        E[:, H:2 * H] += bhh[H:2 * H]
        return E
    Ef3 = make_E(Wih_f, bih_f, bhh_f)
    Eb3 = make_E(Wih_b, bih_b, bhh_b)
    Ef = Ef3[:, 0:2 * H].astype(bf)
    Eb = Eb3[:, 0:2 * H].astype(bf)
    EfN = Ef3[:, 2 * H:3 * H].astype(bf)
    EbN = Eb3[:, 2 * H:3 * H].astype(bf)

    def step_block(c, gi, t):
        L = schedule[gi]
        NW = NWs[gi]
        W2 = 2 * NW
        idx = ids[gi][c].clip(0)
        ch = chars[idx]
        code_f = ch[:, t]
        code_b = ch[:, L - 1 - t]
        blk = np.zeros((VOCAB, 2 * W2), dtype=bf)
        ar = np.arange(NW)
        blk[code_f, ar] = 1
        blk[code_b, NW + ar] = 1
        blk[:, W2:W2 + NW] = EfN[code_f].T
        blk[:, W2 + NW:2 * W2] = EbN[code_b].T
        return blk

    gis_ = list(range(len(schedule)))
    pairs = [(gis_[i], gis_[i + 1]) if i + 1 < len(schedule) else (gis_[i], None)
             for i in range(0, len(schedule), 2)]
    combs = []
    for c in range(NCORES):
        blocks = []
        for gA, gB in pairs:
            LA = schedule[gA]
            LB = schedule[gB] if gB is not None else 0
            for t in range(max(LA, LB)):
                if t < LA:
                    blocks.append(step_block(c, gA, t))
                if t < LB:
                    blocks.append(step_block(c, gB, t))
        combs.append(np.concatenate(blocks, axis=1))

    wpt = Wp.T
    wpt2 = np.concatenate([wpt[0:H, :], wpt[H:2 * H, :]], axis=1)
    s0 = float(ctx[:, 0] @ np.tanh(bp))
    es0 = float(np.exp(s0))

    params = dict(
        ef=Ef, eb=Eb,
        wtf=Whh_f.T.astype(bf), wtb=Whh_b.T.astype(bf),
        wpt=wpt2.astype(bf),
        ctxv=ctx.astype(bf).reshape(C, 1), bp=bp.astype(np.float32).reshape(C, 1),
        bhnfc=bhh_f[2 * H:].astype(np.float32).reshape(H, 1),
        bhnbc=bhh_b[2 * H:].astype(np.float32).reshape(H, 1),
        iden=np.eye(H, dtype=bf),
    )
    in_maps = [dict(comb=combs[c], **params) for c in range(NCORES)]
    return schedule, NWs, in_maps, ids, es0


def post_host_v4(results, schedule, NWs, ids, es0, B):
    acc_offs = np.cumsum([0] + [2 * nw for nw in NWs]).tolist()
    out = np.zeros((B, 2 * H), dtype=np.float32)
    for c in range(NCORES):
        acc = np.asarray(results[c]["acc"], dtype=np.float32)
        den = np.asarray(results[c]["den"], dtype=np.float32)
        for gi, L in enumerate(schedule):
            NW = NWs[gi]
            idlist = ids[gi][c]
            w = np.nonzero(idlist >= 0)[0]
            if len(w) == 0:
                continue
            denw = np.empty(NW, dtype=np.float32)
            denw[0:128] = den[:, 2 * gi]
            denw[128:NW] = den[0:NW - 128, 2 * gi + 1]
            denom = denw[w] + (T - L) * es0
            a = acc[:, acc_offs[gi]:acc_offs[gi] + 2 * NW]
            out[idlist[w], 0:H] = (a[:, w] / denom).T
            out[idlist[w], H:2 * H] = (a[:, NW + w] / denom).T
    return out


_NC_CACHE = {}


def get_nc(schedule, NWs, repeat=1):
    key = (tuple(schedule), tuple(NWs), repeat)
    if key not in _NC_CACHE:
        _NC_CACHE[key] = build_nc_v4(schedule, NWs, repeat=repeat)
    return _NC_CACHE[key]


def kernel(**inputs):
    chars = np.asarray(inputs['padded_char_tensor'])
    lens = np.asarray(inputs['sequence_lens'])
    B = chars.shape[0]
    args = [np.asarray(inputs[k], dtype=np.float32) for k in (
        'emb', 'Wih_f', 'Whh_f', 'bih_f', 'bhh_f',
        'Wih_b', 'Whh_b', 'bih_b', 'bhh_b', 'Wp', 'bp', 'ctx')]
    schedule, NWs, in_maps, ids, es0 = prep_host_v4(
        chars.astype(np.int64), lens.astype(np.int64), *args)
    nc = get_nc(schedule, NWs)
    from concourse.bass_utils import run_bass_kernel_spmd
    res = run_bass_kernel_spmd(nc, in_maps, list(range(NCORES)))
    out = post_host_v4(res.results, schedule, NWs, ids, es0, B)
    return out.astype(np.float32)

